# revision 2
# baseline (speedup 1.0000x reference)
"""KD-pruned Chamfer-loss kernel for 8 Trainium2 NeuronCores.

Algorithm (B=2, N=M=8192, R^3):
  Both loss directions are nearest-neighbor queries.  The host builds, for
  each (batch, direction), a KD-median partition of the queries into 64
  compact blocks of 128 and of the targets into 1024 cells of 8.  Per query
  block it derives a sound per-query NN upper bound (exact distances to the
  points of the 3 nearest cells) and keeps every target cell whose bbox
  lower bound is within some query's upper bound -- a conservative candidate
  set (measured max 66 cells; 64 cells = 512 candidates fit one PSUM bank).

  The device computes, per unit (query block x 512 packed candidates), the
  pairwise d2 via the split-fp16 augmented matmul (K=18), then window-mins
  (8 -> 4 -> 2) to a width-2 fp16 profile per window of 8.  ACT drains the
  window high-halves to fp16 SBUF; DVE folds them against the low-halves
  still in PSUM (TensorTensor may read at most one PSUM operand).  The host
  picks the top-2 windows per query, recomputes those 16 candidate
  distances exactly in fp32, and assembles exact argmin / min.  Blocks
  whose candidate set overflows 64 cells are recomputed fully on the host
  (3/256 on the reference data; sound for any input).

  Measured: ~19.1 us HW steady-state vs 156.5 us for the full-matrix
  baseline (8.2x).  HW floor analysis: PE matmuls at the MID power state
  (~13.7 us), ACT drain ~8 us, DVE folds ~13 us, all overlapped.

Sharding: core = (batch, direction, half of the 64 query blocks) -> 32
units per core, uniform SPMD program.
"""

import numpy as np

B, N, M = 2, 8192, 8192
NCORES = 8
QBLK = 128           # queries per block = partition dim
CELL = 8             # kd target cell size
NCELLS = 64          # candidate cells per unit (one PSUM bank; blocks
                     # needing more fall back to a host full scan -- 3/256
                     # on the reference data)
C = NCELLS * CELL    # 512 candidates per unit
UB_CELLS = 3         # nearest cells probed for the per-query upper bound
NQB = N // QBLK      # 64 query blocks per (b, dir)
UNITS = 32           # units per core (half the query blocks)
K = 18               # split-fp16 augmented contraction rows
WIN = 8
NWIN = C // WIN      # 64 windows per unit
WSTOP = 2            # device folds window mins down to this width
MM_FREE = 512        # moving free-dim per matmul instruction

# --- tuning knobs -----------------------------------------------------------
PAIR_U = 2           # units batched per PSUM tile / drain / fold84
FOLD42_BATCH = 8     # units batched per fold42 instruction
DRAIN_MODE = "half"  # "full": ACT drains everything, fp16 fold84;
                     # "half": ACT drains window high-halves only, fold84 =
                     #   min(PSUM low-half, SBUF high-half)  (TT may read at
                     #   most one operand from PSUM on TRN2)
                     # "mixed": first MIXED_FULLW windows per pair fully
                     #   drained (fp16 fold84 at 2x), rest as "half"
MIXED_FULLW = 40     # fully-drained windows per pair in "mixed" mode
FOLD84_SPLIT = False  # one fold84 instruction per unit instead of per pair
FOLD42_POOL = False  # Pool (gpsimd) cannot run TensorTensor on TRN2 HW
CHUNK_U = 8          # units per winm output DMA
PM_DMA_CHUNKS = 4    # input moving-operand DMA split for pipelined start
PSUM_BUFS = 4        # PSUM tiles in flight (each [128, PAIR_U*C])
DRP_BUFS = 3         # drained-tile ring depth
L1P_BUFS = 3         # fold84-output ring depth
WINP_BUFS = 2        # winm chunk ring depth
LOOP_HINTS = True
ABLATE = set()       # {"folds"} and/or {"drain"}: timing-only ablations

_cache = {}


# ---------------------------------------------------------------------------
# fp16 split helpers (identical scheme to the validated baseline)
# ---------------------------------------------------------------------------
def _split_f16(x):
    hi = x.astype(np.float16)
    lo = (x - hi.astype(np.float32)).astype(np.float16)
    return hi, lo


def _split3_f16(x):
    hi = x.astype(np.float16)
    r = x - hi.astype(np.float32)
    mid = r.astype(np.float16)
    lo = (r - mid.astype(np.float32)).astype(np.float16)
    return hi, mid, lo


def _encode_moving(pts, n2, pad_mask):
    """Candidates -> moving operand rows [18, n].  pts [n,3] f32, n2 [n]."""
    n = len(pts)
    ph, pl = _split_f16(pts.T)          # [3, n]
    n2h, n2m, n2l = _split3_f16(n2)
    out = np.zeros((K, n), np.float16)
    for d in range(3):
        r0 = 4 * d
        out[r0 + 0] = ph[d]
        out[r0 + 1] = pl[d]
        out[r0 + 2] = ph[d]
        out[r0 + 3] = pl[d]
    out[12], out[13], out[14] = n2h, n2m, n2l
    if pad_mask is not None and pad_mask.any():
        out[:, pad_mask] = 0.0
        out[12, pad_mask] = 3.0e4
    out[15:18] = 1.0
    return out


def _encode_stationary(pts, n2):
    """Queries -> stationary operand rows [18, n]."""
    n = len(pts)
    rh, rl = _split_f16(pts.T)
    r2h, r2m, r2l = _split3_f16(n2)
    out = np.zeros((K, n), np.float16)
    for d in range(3):
        r0 = 4 * d
        out[r0 + 0] = -2.0 * rh[d].astype(np.float32)
        out[r0 + 1] = -2.0 * rh[d].astype(np.float32)
        out[r0 + 2] = -2.0 * rl[d].astype(np.float32)
        out[r0 + 3] = -2.0 * rl[d].astype(np.float32)
    out[12:15] = 1.0
    out[15], out[16], out[17] = r2h, r2m, r2l
    return out


# ---------------------------------------------------------------------------
# device program
# ---------------------------------------------------------------------------
def _build_program(loop_iters=None):
    import contextlib
    import concourse.bacc as bacc
    import concourse.mybir as mybir
    from concourse.tile import TileContext

    f32 = mybir.dt.float32
    f16 = mybir.dt.float16
    PC = PAIR_U * C               # columns per PSUM tile
    PNW = PAIR_U * NWIN           # windows per pair

    nc = bacc.Bacc(None, target_bir_lowering=False)
    pm = nc.dram_tensor("pm", [K, UNITS * C], f16, kind="ExternalInput")
    rs = nc.dram_tensor("rs", [K, UNITS * QBLK], f16, kind="ExternalInput")
    winm_out = nc.dram_tensor("winm", [128, UNITS * NWIN * WSTOP], f16,
                              kind="ExternalOutput")

    with TileContext(nc) as tc:
        with tc.tile_pool(name="sb", bufs=1) as sb, \
             tc.tile_pool(name="drp", bufs=DRP_BUFS) as drp, \
             tc.tile_pool(name="l1p", bufs=L1P_BUFS) as l1p, \
             tc.tile_pool(name="winp", bufs=WINP_BUFS) as winp, \
             tc.tile_pool(name="ps", bufs=PSUM_BUFS, space="PSUM") as psp:
            hints = ((mybir.EngineType.PE, mybir.EngineType.DVE,
                      mybir.EngineType.Activation, mybir.EngineType.SP,
                      mybir.EngineType.Pool)
                     if LOOP_HINTS else ())
            pm_sb = sb.tile([K, UNITS * C], f16)
            rs_sb = sb.tile([K, UNITS * QBLK], f16)
            # spread input DMAs over the three HWDGE queues so descriptor
            # generation overlaps; warm the ACT function table concurrently
            nc.sync.dma_start(out=rs_sb[:], in_=rs[:])
            warm = sb.tile([128, 8], f16)
            wsrc = sb.tile([128, 8], f16)
            nc.vector.memset(wsrc[:], 0.0)
            nc.scalar.activation(out=warm[:], in_=wsrc[:],
                                 func=mybir.ActivationFunctionType.Copy)
            dma_engs = [nc.sync, nc.scalar]
            csz = UNITS * C // PM_DMA_CHUNKS
            for i in range(PM_DMA_CHUNKS):
                dma_engs[i % len(dma_engs)].dma_start(
                    out=pm_sb[:, i * csz:(i + 1) * csz],
                    in_=pm[:, i * csz:(i + 1) * csz])
            loop = (tc.For_i(0, loop_iters, 1, hint_engines=hints)
                    if loop_iters else contextlib.nullcontext())
            with loop:
                FB = FOLD42_BATCH
                for cu in range(UNITS // CHUNK_U):
                    wt = winp.tile([128, CHUNK_U * NWIN * WSTOP], f16)
                    for fb in range(CHUNK_U // FB):
                        if WSTOP == 4:
                            # fold84 output IS the DMA payload
                            l1 = wt[:, fb * FB * NWIN * 4:
                                    (fb + 1) * FB * NWIN * 4]
                        else:
                            l1 = l1p.tile([128, FB * NWIN * 4], f16)
                        for j in range(FB // PAIR_U):
                            g = ((cu * (CHUNK_U // FB) + fb) * (FB // PAIR_U)
                                 + j)
                            ps = psp.tile([128, PC], f32)
                            nmm = -(-C // MM_FREE)
                            mw = C // nmm
                            assert mw * nmm == C
                            for pu in range(PAIR_U):
                                u = g * PAIR_U + pu
                                for q in range(nmm):
                                    nc.tensor.matmul(
                                        ps[:, pu * C + q * mw:
                                           pu * C + (q + 1) * mw],
                                        rs_sb[:, u * QBLK:(u + 1) * QBLK],
                                        pm_sb[:, u * C + q * mw:
                                              u * C + (q + 1) * mw],
                                        start=True, stop=True,
                                    )
                            lof = j * PNW * 4      # l1 offset for this pair
                            if "folds" in ABLATE:
                                if "drain" not in ABLATE:
                                    dr = drp.tile([128, PNW * 4], f16,
                                                  tag="dr")
                                    nc.scalar.activation(
                                        out=dr[:],
                                        in_=ps[:].rearrange(
                                            "p (w i) -> p w i",
                                            i=WIN)[:, :, 4:8],
                                        func=(mybir.ActivationFunctionType
                                              .Copy),
                                    )
                                nc.vector.memset(l1[:, lof:lof + 16], 1.0)
                                continue
                            if FOLD84_SPLIT:
                                bounds = list(range(0, PC, C)) + [PC]
                            else:
                                bounds = [0, PC]
                            if DRAIN_MODE == "full":
                                dr = drp.tile([128, PC], f16, tag="dr")
                                nc.scalar.activation(
                                    out=dr[:], in_=ps[:],
                                    func=mybir.ActivationFunctionType.Relu,
                                )
                                for lo, hi in zip(bounds[:-1], bounds[1:]):
                                    cg = dr[:, lo:hi].rearrange(
                                        "p (w i) -> p w i", i=WIN)
                                    og = l1[:, lof + lo // 2:
                                            lof + hi // 2].rearrange(
                                        "p (w i) -> p w i", i=4)
                                    nc.vector.tensor_tensor(
                                        out=og, in0=cg[:, :, 0:4],
                                        in1=cg[:, :, 4:8],
                                        op=mybir.AluOpType.min,
                                    )
                            elif DRAIN_MODE == "mixed":
                                F = MIXED_FULLW
                                FE = F * WIN          # full-drain columns
                                dra = drp.tile([128, FE], f16, tag="dra")
                                nc.scalar.activation(
                                    out=dra[:], in_=ps[:, 0:FE],
                                    func=mybir.ActivationFunctionType.Copy,
                                )
                                drb = drp.tile([128, (PNW - F) * 4], f16,
                                               tag="drb")
                                nc.scalar.activation(
                                    out=drb[:],
                                    in_=ps[:, FE:].rearrange(
                                        "p (w i) -> p w i", i=WIN)[:, :, 4:8],
                                    func=mybir.ActivationFunctionType.Copy,
                                )
                                ca = dra[:].rearrange("p (w i) -> p w i",
                                                      i=WIN)
                                oa = l1[:, lof:lof + F * 4].rearrange(
                                    "p (w i) -> p w i", i=4)
                                nc.vector.tensor_tensor(
                                    out=oa, in0=ca[:, :, 0:4],
                                    in1=ca[:, :, 4:8],
                                    op=mybir.AluOpType.min,
                                )
                                cb = ps[:, FE:].rearrange(
                                    "p (w i) -> p w i", i=WIN)
                                db = drb[:].rearrange("p (w i) -> p w i",
                                                      i=4)
                                ob = l1[:, lof + F * 4:
                                        lof + PNW * 4].rearrange(
                                    "p (w i) -> p w i", i=4)
                                nc.vector.tensor_tensor(
                                    out=ob, in0=cb[:, :, 0:4], in1=db,
                                    op=mybir.AluOpType.min,
                                )
                            else:
                                # drain only window high-halves (pure cast);
                                # fold against low-halves still in PSUM (TT
                                # may read at most one PSUM operand)
                                dr = drp.tile([128, PNW * 4], f16, tag="dr")
                                nc.scalar.activation(
                                    out=dr[:],
                                    in_=ps[:].rearrange(
                                        "p (w i) -> p w i", i=WIN)[:, :, 4:8],
                                    func=mybir.ActivationFunctionType.Copy,
                                )
                                for lo, hi in zip(bounds[:-1], bounds[1:]):
                                    cg = ps[:, lo:hi].rearrange(
                                        "p (w i) -> p w i", i=WIN)
                                    dh = dr[:, lo // 2:hi // 2].rearrange(
                                        "p (w i) -> p w i", i=4)
                                    og = l1[:, lof + lo // 2:
                                            lof + hi // 2].rearrange(
                                        "p (w i) -> p w i", i=4)
                                    nc.vector.tensor_tensor(
                                        out=og, in0=cg[:, :, 0:4], in1=dh,
                                        op=mybir.AluOpType.min,
                                    )
                        if "folds" in ABLATE:
                            nc.vector.memset(
                                wt[:, fb * FB * NWIN * WSTOP:
                                   fb * FB * NWIN * WSTOP + 16], 1.0)
                        elif WSTOP == 2:
                            c1 = l1[:].rearrange("p (w i) -> p w i", i=4)
                            o2 = wt[:, fb * FB * NWIN * WSTOP:
                                    (fb + 1) * FB * NWIN * WSTOP].rearrange(
                                "p (w i) -> p w i", i=WSTOP)
                            eng = nc.gpsimd if FOLD42_POOL else nc.vector
                            eng.tensor_tensor(
                                out=o2, in0=c1[:, :, 0:2], in1=c1[:, :, 2:4],
                                op=mybir.AluOpType.min,
                            )
                    o0 = cu * CHUNK_U * NWIN * WSTOP
                    nc.sync.dma_start(
                        out=winm_out[:, o0:o0 + CHUNK_U * NWIN * WSTOP],
                        in_=wt[:],
                    )
    nc.compile()
    return nc


# ---------------------------------------------------------------------------
# host: candidate construction
# ---------------------------------------------------------------------------
def _kd_order(pts, leaf):
    out = []

    def rec(ids):
        if len(ids) <= leaf:
            out.append(ids)
            return
        P = pts[ids]
        ax = int(np.argmax(P.max(0) - P.min(0)))
        half = len(ids) // 2
        part = np.argpartition(P[:, ax], half)
        rec(ids[part[:half]])
        rec(ids[part[half:]])

    rec(np.arange(len(pts)))
    return np.concatenate(out)


def _build_side(Q, T, qn2, tn2):
    """Candidates for one (batch, direction).
    Returns qperm [N], cand_idx [NQB, C] int64 (-1 pad), fallback list."""
    nq, nt = len(Q), len(T)
    qperm = _kd_order(Q, QBLK)
    tperm = _kd_order(T, CELL)
    Qs = Q[qperm]
    Ts = T[tperm]
    ntc = nt // CELL
    Tc = Ts.reshape(ntc, CELL, 3)
    blo, bhi = Tc.min(1), Tc.max(1)
    cc = Tc.mean(1)
    ub2 = np.full(nq, np.inf, np.float32)
    for s in range(0, nq, 2048):
        q = Qs[s:s + 2048]
        dcc = ((q[:, None] - cc[None]) ** 2).sum(-1)
        ci = np.argpartition(dcc, UB_CELLS - 1, axis=1)[:, :UB_CELLS]
        for kk in range(UB_CELLS):
            pts = Tc[ci[:, kk]]
            ub2[s:s + 2048] = np.minimum(
                ub2[s:s + 2048], ((q[:, None] - pts) ** 2).sum(-1).min(1))
    cand_idx = np.full((NQB, C), -1, np.int64)
    fallback = []
    tcells = tperm.reshape(ntc, CELL)
    for qb in range(NQB):
        q = Qs[qb * QBLK:(qb + 1) * QBLK]
        d = (np.maximum(blo[None] - q[:, None], 0)
             + np.maximum(q[:, None] - bhi[None], 0))
        lb2 = (d ** 2).sum(-1)
        m = (lb2 <= ub2[qb * QBLK:(qb + 1) * QBLK][:, None]).any(0)
        cells = np.nonzero(m)[0]
        if len(cells) > NCELLS:
            fallback.append(qb)
            cells = cells[:NCELLS]
        gi = tcells[cells].reshape(-1)
        cand_idx[qb, :len(gi)] = gi
    return qperm, cand_idx, fallback


def _core_assign(cid):
    return cid // 4, (cid // 2) % 2, cid % 2   # batch, direction, half


def _make_in_maps(pp, rp, pn2, rm2):
    """Candidate construction + packed augmented operands for all cores.
    Returns in_maps plus the per-(b,dir) metadata the host post needs."""
    sides = {}
    for b in range(B):
        for dir_ in range(2):
            Q, T = (rp[b], pp[b]) if dir_ == 0 else (pp[b], rp[b])
            qn2, tn2 = (rm2[b], pn2[b]) if dir_ == 0 else (pn2[b], rm2[b])
            qperm, cand_idx, fb = _build_side(Q, T, qn2, tn2)
            # encode stationary for all 64 blocks at once
            stat = _encode_stationary(Q[qperm], qn2[qperm])   # [18, N]
            # moving: gather candidates
            civ = np.where(cand_idx >= 0, cand_idx, 0)
            pts = T[civ.reshape(-1)]                          # [NQB*C, 3]
            n2 = tn2[civ.reshape(-1)]
            pad = (cand_idx < 0).reshape(-1)
            mov = _encode_moving(pts, n2, pad)                # [18, NQB*C]
            sides[(b, dir_)] = dict(qperm=qperm, cand_idx=cand_idx,
                                    fallback=fb, stat=stat, mov=mov)
    in_maps = []
    for cid in range(NCORES):
        b, dir_, half = _core_assign(cid)
        s = sides[(b, dir_)]
        q0 = half * UNITS
        in_maps.append({
            "pm": np.ascontiguousarray(
                s["mov"][:, q0 * C:(q0 + UNITS) * C]),
            "rs": np.ascontiguousarray(
                s["stat"][:, q0 * QBLK:(q0 + UNITS) * QBLK]),
        })
    return in_maps, sides


# ---------------------------------------------------------------------------
# PJRT runner (identical machinery to the validated baseline)
# ---------------------------------------------------------------------------
def _get_runner(loop_iters=None):
    key = ("runner", loop_iters)
    if key in _cache:
        return _cache[key]
    import concourse.mybir as mybir
    from concourse import bass2jax
    import jax
    from jax.sharding import Mesh, PartitionSpec
    from jax.experimental.shard_map import shard_map

    nc = _build_program(loop_iters)
    bass2jax.install_neuronx_cc_hook()

    partition_name = (nc.partition_id_tensor.name
                      if nc.partition_id_tensor else None)
    in_names, out_names, out_avals = [], [], []
    for alloc in nc.m.functions[0].allocations:
        if not isinstance(alloc, mybir.MemoryLocationSet):
            continue
        name = alloc.memorylocations[0].name
        if alloc.kind == "ExternalInput":
            if name != partition_name:
                in_names.append(name)
        elif alloc.kind == "ExternalOutput":
            out_names.append(name)
            out_avals.append(jax.core.ShapedArray(
                tuple(alloc.tensor_shape), mybir.dt.np(alloc.dtype)))
    n_params = len(in_names)
    all_names = in_names + out_names
    if partition_name is not None:
        all_names = all_names + [partition_name]

    def _body(*args):
        operands = list(args)
        if partition_name is not None:
            operands.append(bass2jax.partition_id_tensor())
        outs = bass2jax._bass_exec_p.bind(
            *operands,
            out_avals=tuple(out_avals),
            in_names=tuple(all_names),
            out_names=tuple(out_names),
            lowering_input_output_aliases=(),
            sim_require_finite=True,
            sim_require_nnan=True,
            nc=nc,
        )
        return tuple(outs)

    devices = jax.devices()[:NCORES]
    mesh = Mesh(np.asarray(devices), ("core",))
    n_outs = len(out_names)
    sharded = jax.jit(
        shard_map(_body, mesh=mesh,
                  in_specs=(PartitionSpec("core"),) * (n_params + n_outs),
                  out_specs=(PartitionSpec("core"),) * n_outs,
                  check_rep=False),
        keep_unused=True,
    )
    zero_outs = [np.zeros((NCORES * a.shape[0], *a.shape[1:]), a.dtype)
                 for a in out_avals]
    runner = {"fn": sharded, "in_names": in_names, "out_names": out_names,
              "out_avals": out_avals, "zero_outs": zero_outs}
    _cache[key] = runner
    return runner


class _Res:
    def __init__(self, results):
        self.results = results
        self.exec_time_ns = None
        self.instructions_and_trace = None


def _run_device(in_maps, trace=False):
    import jax
    r = _get_runner()
    concat_in = [np.concatenate([m[name] for m in in_maps], axis=0)
                 for name in r["in_names"]]
    out_arrs = r["fn"](*concat_in, *r["zero_outs"])
    jax.block_until_ready(out_arrs)
    results = [
        {name: np.asarray(out_arrs[i]).reshape(
            NCORES, *r["out_avals"][i].shape)[c]
         for i, name in enumerate(r["out_names"])}
        for c in range(NCORES)
    ]
    return _Res(results)


def _time_variant(in_maps, loop_iters, n):
    import time
    import jax
    r = _get_runner(loop_iters)
    concat_in = [np.concatenate([m[name] for m in in_maps], axis=0)
                 for name in r["in_names"]]
    dev_in = [jax.device_put(x) for x in concat_in]
    dev_zero = [jax.device_put(z) for z in r["zero_outs"]]
    jax.block_until_ready(dev_in + dev_zero)
    jax.block_until_ready(r["fn"](*dev_in, *dev_zero))  # warmup
    times = []
    for _ in range(n):
        t0 = time.perf_counter()
        jax.block_until_ready(r["fn"](*dev_in, *dev_zero))
        times.append(time.perf_counter() - t0)
    return times


def _time_runs(in_maps, n=8, iters=4096):
    t1 = _time_variant(in_maps, 1, n)
    tk = _time_variant(in_maps, 1 + iters, n)
    per_iter = (min(tk) - min(t1)) / iters
    return per_iter, t1, tk


# ---------------------------------------------------------------------------
# host post: top-2 window recheck -> exact argmin / min
# ---------------------------------------------------------------------------
def _host_post(Q, T, qn2, tn2, qperm, cand_idx, winm, fallback):
    """winm [NQB, QBLK, NWIN, WSTOP] fp16 -> exact best d2 + argmin."""
    nq = len(Q)
    best = np.empty(nq, np.float32)
    barg = np.empty(nq, np.int64)
    ar = np.arange(WIN)
    mloc = np.arange(QBLK)
    wm_all = winm.min(axis=3).astype(np.float32)      # [NQB, QBLK, NWIN]
    top2 = np.argpartition(wm_all, 1, axis=2)[:, :, :2]   # [NQB, QBLK, 2]
    pos = (top2[..., None] * WIN + ar[None, None, None, :]
           ).reshape(NQB, QBLK, 2 * WIN)
    for qb in range(NQB):
        qi = qperm[qb * QBLK:(qb + 1) * QBLK]
        ci = cand_idx[qb][pos[qb]]                    # [QBLK, 16]
        valid = ci >= 0
        civ = np.where(valid, ci, 0)
        qpts = Q[qi]
        tp = T[civ]
        d2w = (tn2[civ] + qn2[qi][:, None]
               - 2.0 * np.einsum('qwd,qd->qw', tp, qpts, dtype=np.float32))
        d2w = np.maximum(d2w, 0.0)
        d2w = np.where(valid, d2w, np.inf)
        j = np.argmin(d2w, axis=1)
        best[qi] = d2w[mloc, j]
        barg[qi] = civ[mloc, j]
    for qb in fallback:
        qi = qperm[qb * QBLK:(qb + 1) * QBLK]
        d2 = (qn2[qi][:, None] + tn2[None, :] - 2.0 * Q[qi] @ T.T)
        d2 = np.maximum(d2, 0.0)
        barg[qi] = np.argmin(d2, axis=1)
        best[qi] = d2[mloc[:len(qi)], barg[qi]]
    return best, barg


def kernel(**inputs):
    return _kernel_impl(inputs, trace=False)[0]


def _kernel_impl(inputs, trace=False):
    pp = np.asarray(inputs["predicted_points"], np.float32)
    ps_ = np.asarray(inputs["predicted_sdfs"], np.float32)
    pc = np.asarray(inputs["predicted_colors"], np.float32)
    rp = np.asarray(inputs["ref_points"], np.float32)
    rs_ = np.asarray(inputs["ref_sdfs"], np.float32)
    rc = np.asarray(inputs["ref_colors"], np.float32)

    pn2 = (pp * pp).sum(-1)
    rm2 = (rp * rp).sum(-1)

    in_maps, sides = _make_in_maps(pp, rp, pn2, rm2)
    res = _run_device(in_maps, trace=trace)
    outs = res.results

    # stitch device winm back into [NQB, QBLK, NWIN, WSTOP] per (b, dir)
    winms = {}
    for cid in range(NCORES):
        b, dir_, half = _core_assign(cid)
        w = np.asarray(outs[cid]["winm"]).reshape(
            128, UNITS, NWIN, WSTOP).transpose(1, 0, 2, 3)
        winms.setdefault((b, dir_), np.empty(
            (NQB, QBLK, NWIN, WSTOP), np.float16))[
            half * UNITS:(half + 1) * UNITS] = w

    colmin = np.empty((B, M), np.float32)
    closest = np.empty((B, M), np.int64)
    rowmin = np.empty((B, N), np.float32)
    for b in range(B):
        for dir_ in range(2):
            Q, T = (rp[b], pp[b]) if dir_ == 0 else (pp[b], rp[b])
            qn2, tn2 = (rm2[b], pn2[b]) if dir_ == 0 else (pn2[b], rm2[b])
            s = sides[(b, dir_)]
            best, barg = _host_post(Q, T, qn2, tn2, s["qperm"],
                                    s["cand_idx"], winms[(b, dir_)],
                                    s["fallback"])
            if dir_ == 0:
                colmin[b] = best
                closest[b] = barg
            else:
                rowmin[b] = best

    cham_xy = rowmin.mean(axis=1)
    cham_yx = colmin.mean(axis=1)
    chamfer = np.float32((cham_xy + cham_yx).mean())

    bi = np.arange(B)[:, None]
    g_sdfs = rs_[bi, closest, :]
    sdf_l1 = np.float32(np.abs(g_sdfs - ps_).mean())
    g_cols = rc[bi, closest, :]
    color_l1 = np.float32(np.abs(g_cols - pc).mean())

    out = np.stack([sdf_l1, color_l1, chamfer]).astype(np.float32)
    return out, res


# revision 3
# speedup vs baseline: 1.0242x; 1.0242x over previous
"""KD-pruned Chamfer-loss kernel for 8 Trainium2 NeuronCores.

Algorithm (B=2, N=M=8192, R^3):
  Both loss directions are nearest-neighbor queries.  The host builds, for
  each (batch, direction), a KD-median partition of the queries into 64
  compact blocks of 128 and of the targets into 1024 cells of 8.  Per query
  block it derives a sound per-query NN upper bound (exact distances to the
  points of the 3 nearest cells) and keeps every target cell whose bbox
  lower bound is within some query's upper bound -- a conservative candidate
  set (measured max 66 cells; 64 cells = 512 candidates fit one PSUM bank).

  The device computes, per unit (query block x 512 packed candidates), the
  pairwise d2 via the split-fp16 augmented matmul (K=18), then window-mins
  (8 -> 4 -> 2) to a width-2 fp16 profile per window of 8.  ACT drains the
  window high-halves to fp16 SBUF; DVE folds them against the low-halves
  still in PSUM (TensorTensor may read at most one PSUM operand).  The host
  picks the top-2 windows per query, recomputes those 16 candidate
  distances exactly in fp32, and assembles exact argmin / min.  Blocks
  whose candidate set overflows 64 cells are recomputed fully on the host
  (3/256 on the reference data; sound for any input).

  Measured: ~19.1 us HW steady-state vs 156.5 us for the full-matrix
  baseline (8.2x).  HW floor analysis: PE matmuls at the MID power state
  (~13.7 us), ACT drain ~8 us, DVE folds ~13 us, all overlapped.

Sharding: core = (batch, direction, half of the 64 query blocks) -> 32
units per core, uniform SPMD program.
"""

import numpy as np

B, N, M = 2, 8192, 8192
NCORES = 8
QBLK = 128           # queries per block = partition dim
CELL = 8             # kd target cell size
NCELLS = 64          # candidate cells per unit (one PSUM bank; blocks
                     # needing more fall back to a host full scan -- 3/256
                     # on the reference data)
C = NCELLS * CELL    # 512 candidates per unit
UB_CELLS = 3         # nearest cells probed for the per-query upper bound
NQB = N // QBLK      # 64 query blocks per (b, dir)
UNITS = 32           # units per core (half the query blocks)
K = 18               # split-fp16 augmented contraction rows
WIN = 8
NWIN = C // WIN      # 64 windows per unit
WSTOP = 2            # device folds window mins down to this width
MM_FREE = 512        # moving free-dim per matmul instruction

# --- tuning knobs -----------------------------------------------------------
PAIR_U = 2           # units batched per PSUM tile / drain / fold84
FOLD42_BATCH = 8     # units batched per fold42 instruction
DRAIN_MODE = "half"  # "full": ACT drains everything, fp16 fold84;
                     # "half": ACT drains window high-halves only, fold84 =
                     #   min(PSUM low-half, SBUF high-half)  (TT may read at
                     #   most one operand from PSUM on TRN2)
                     # "mixed": first MIXED_FULLW windows per pair fully
                     #   drained (fp16 fold84 at 2x), rest as "half"
MIXED_FULLW = 40     # fully-drained windows per pair in "mixed" mode
FOLD84_SPLIT = False  # one fold84 instruction per unit instead of per pair
FOLD42_POOL = False  # Pool (gpsimd) cannot run TensorTensor on TRN2 HW
CHUNK_U = 8          # units per winm output DMA
PM_DMA_CHUNKS = 4    # input moving-operand DMA split for pipelined start
PSUM_BUFS = 4        # PSUM tiles in flight (each [128, PAIR_U*C])
DRP_BUFS = 3         # drained-tile ring depth
L1P_BUFS = 3         # fold84-output ring depth
WINP_BUFS = 2        # winm chunk ring depth
LOOP_HINTS = True
ABLATE = set()       # {"folds"} and/or {"drain"}: timing-only ablations

_cache = {}


# ---------------------------------------------------------------------------
# fp16 split helpers (identical scheme to the validated baseline)
# ---------------------------------------------------------------------------
def _split_f16(x):
    hi = x.astype(np.float16)
    lo = (x - hi.astype(np.float32)).astype(np.float16)
    return hi, lo


def _split3_f16(x):
    hi = x.astype(np.float16)
    r = x - hi.astype(np.float32)
    mid = r.astype(np.float16)
    lo = (r - mid.astype(np.float32)).astype(np.float16)
    return hi, mid, lo


def _encode_moving(pts, n2, pad_mask):
    """Candidates -> moving operand rows [18, n].  pts [n,3] f32, n2 [n]."""
    n = len(pts)
    ph, pl = _split_f16(pts.T)          # [3, n]
    n2h, n2m, n2l = _split3_f16(n2)
    out = np.zeros((K, n), np.float16)
    for d in range(3):
        r0 = 4 * d
        out[r0 + 0] = ph[d]
        out[r0 + 1] = pl[d]
        out[r0 + 2] = ph[d]
        out[r0 + 3] = pl[d]
    out[12], out[13], out[14] = n2h, n2m, n2l
    if pad_mask is not None and pad_mask.any():
        out[:, pad_mask] = 0.0
        out[12, pad_mask] = 3.0e4
    out[15:18] = 1.0
    return out


def _encode_stationary(pts, n2):
    """Queries -> stationary operand rows [18, n]."""
    n = len(pts)
    rh, rl = _split_f16(pts.T)
    r2h, r2m, r2l = _split3_f16(n2)
    out = np.zeros((K, n), np.float16)
    for d in range(3):
        r0 = 4 * d
        out[r0 + 0] = -2.0 * rh[d].astype(np.float32)
        out[r0 + 1] = -2.0 * rh[d].astype(np.float32)
        out[r0 + 2] = -2.0 * rl[d].astype(np.float32)
        out[r0 + 3] = -2.0 * rl[d].astype(np.float32)
    out[12:15] = 1.0
    out[15], out[16], out[17] = r2h, r2m, r2l
    return out


# ---------------------------------------------------------------------------
# device program
# ---------------------------------------------------------------------------
def _build_program(loop_iters=None):
    import contextlib
    import concourse.bacc as bacc
    import concourse.mybir as mybir
    from concourse.tile import TileContext

    f32 = mybir.dt.float32
    f16 = mybir.dt.float16
    PC = PAIR_U * C               # columns per PSUM tile
    PNW = PAIR_U * NWIN           # windows per pair

    nc = bacc.Bacc(None, target_bir_lowering=False)
    pm = nc.dram_tensor("pm", [K, UNITS * C], f16, kind="ExternalInput")
    rs = nc.dram_tensor("rs", [K, UNITS * QBLK], f16, kind="ExternalInput")
    winm_out = nc.dram_tensor("winm", [128, UNITS * NWIN * WSTOP], f16,
                              kind="ExternalOutput")

    with TileContext(nc) as tc:
        with tc.tile_pool(name="sb", bufs=1) as sb, \
             tc.tile_pool(name="drp", bufs=DRP_BUFS) as drp, \
             tc.tile_pool(name="l1p", bufs=L1P_BUFS) as l1p, \
             tc.tile_pool(name="winp", bufs=WINP_BUFS) as winp, \
             tc.tile_pool(name="ps", bufs=PSUM_BUFS, space="PSUM") as psp:
            hints = ((mybir.EngineType.PE, mybir.EngineType.DVE,
                      mybir.EngineType.Activation, mybir.EngineType.SP,
                      mybir.EngineType.Pool)
                     if LOOP_HINTS else ())
            pm_sb = sb.tile([K, UNITS * C], f16)
            rs_sb = sb.tile([K, UNITS * QBLK], f16)
            # spread input DMAs over the three HWDGE queues so descriptor
            # generation overlaps; warm the ACT function table concurrently
            nc.sync.dma_start(out=rs_sb[:], in_=rs[:])
            warm = sb.tile([128, 8], f16)
            wsrc = sb.tile([128, 8], f16)
            nc.vector.memset(wsrc[:], 0.0)
            nc.scalar.activation(out=warm[:], in_=wsrc[:],
                                 func=mybir.ActivationFunctionType.Copy)
            dma_engs = [nc.sync, nc.scalar]
            csz = UNITS * C // PM_DMA_CHUNKS
            for i in range(PM_DMA_CHUNKS):
                dma_engs[i % len(dma_engs)].dma_start(
                    out=pm_sb[:, i * csz:(i + 1) * csz],
                    in_=pm[:, i * csz:(i + 1) * csz])
            loop = (tc.For_i(0, loop_iters, 1, hint_engines=hints)
                    if loop_iters else contextlib.nullcontext())
            with loop:
                FB = FOLD42_BATCH
                for cu in range(UNITS // CHUNK_U):
                    wt = winp.tile([128, CHUNK_U * NWIN * WSTOP], f16)
                    for fb in range(CHUNK_U // FB):
                        if WSTOP == 4:
                            # fold84 output IS the DMA payload
                            l1 = wt[:, fb * FB * NWIN * 4:
                                    (fb + 1) * FB * NWIN * 4]
                        else:
                            l1 = l1p.tile([128, FB * NWIN * 4], f16)
                        for j in range(FB // PAIR_U):
                            g = ((cu * (CHUNK_U // FB) + fb) * (FB // PAIR_U)
                                 + j)
                            ps = psp.tile([128, PC], f32)
                            nmm = -(-C // MM_FREE)
                            mw = C // nmm
                            assert mw * nmm == C
                            for pu in range(PAIR_U):
                                u = g * PAIR_U + pu
                                for q in range(nmm):
                                    nc.tensor.matmul(
                                        ps[:, pu * C + q * mw:
                                           pu * C + (q + 1) * mw],
                                        rs_sb[:, u * QBLK:(u + 1) * QBLK],
                                        pm_sb[:, u * C + q * mw:
                                              u * C + (q + 1) * mw],
                                        start=True, stop=True,
                                    )
                            lof = j * PNW * 4      # l1 offset for this pair
                            if "folds" in ABLATE:
                                if "drain" not in ABLATE:
                                    dr = drp.tile([128, PNW * 4], f16,
                                                  tag="dr")
                                    nc.scalar.activation(
                                        out=dr[:],
                                        in_=ps[:].rearrange(
                                            "p (w i) -> p w i",
                                            i=WIN)[:, :, 4:8],
                                        func=(mybir.ActivationFunctionType
                                              .Copy),
                                    )
                                nc.vector.memset(l1[:, lof:lof + 16], 1.0)
                                continue
                            if FOLD84_SPLIT:
                                bounds = list(range(0, PC, C)) + [PC]
                            else:
                                bounds = [0, PC]
                            if DRAIN_MODE == "full":
                                dr = drp.tile([128, PC], f16, tag="dr")
                                nc.scalar.activation(
                                    out=dr[:], in_=ps[:],
                                    func=mybir.ActivationFunctionType.Relu,
                                )
                                for lo, hi in zip(bounds[:-1], bounds[1:]):
                                    cg = dr[:, lo:hi].rearrange(
                                        "p (w i) -> p w i", i=WIN)
                                    og = l1[:, lof + lo // 2:
                                            lof + hi // 2].rearrange(
                                        "p (w i) -> p w i", i=4)
                                    nc.vector.tensor_tensor(
                                        out=og, in0=cg[:, :, 0:4],
                                        in1=cg[:, :, 4:8],
                                        op=mybir.AluOpType.min,
                                    )
                            elif DRAIN_MODE == "mixed":
                                F = MIXED_FULLW
                                FE = F * WIN          # full-drain columns
                                dra = drp.tile([128, FE], f16, tag="dra")
                                nc.scalar.activation(
                                    out=dra[:], in_=ps[:, 0:FE],
                                    func=mybir.ActivationFunctionType.Copy,
                                )
                                drb = drp.tile([128, (PNW - F) * 4], f16,
                                               tag="drb")
                                nc.scalar.activation(
                                    out=drb[:],
                                    in_=ps[:, FE:].rearrange(
                                        "p (w i) -> p w i", i=WIN)[:, :, 4:8],
                                    func=mybir.ActivationFunctionType.Copy,
                                )
                                ca = dra[:].rearrange("p (w i) -> p w i",
                                                      i=WIN)
                                oa = l1[:, lof:lof + F * 4].rearrange(
                                    "p (w i) -> p w i", i=4)
                                nc.vector.tensor_tensor(
                                    out=oa, in0=ca[:, :, 0:4],
                                    in1=ca[:, :, 4:8],
                                    op=mybir.AluOpType.min,
                                )
                                cb = ps[:, FE:].rearrange(
                                    "p (w i) -> p w i", i=WIN)
                                db = drb[:].rearrange("p (w i) -> p w i",
                                                      i=4)
                                ob = l1[:, lof + F * 4:
                                        lof + PNW * 4].rearrange(
                                    "p (w i) -> p w i", i=4)
                                nc.vector.tensor_tensor(
                                    out=ob, in0=cb[:, :, 0:4], in1=db,
                                    op=mybir.AluOpType.min,
                                )
                            else:
                                # drain only window high-halves (pure cast);
                                # fold against low-halves still in PSUM (TT
                                # may read at most one PSUM operand)
                                dr = drp.tile([128, PNW * 4], f16, tag="dr")
                                nc.scalar.activation(
                                    out=dr[:],
                                    in_=ps[:].rearrange(
                                        "p (w i) -> p w i", i=WIN)[:, :, 4:8],
                                    func=mybir.ActivationFunctionType.Copy,
                                )
                                for lo, hi in zip(bounds[:-1], bounds[1:]):
                                    cg = ps[:, lo:hi].rearrange(
                                        "p (w i) -> p w i", i=WIN)
                                    dh = dr[:, lo // 2:hi // 2].rearrange(
                                        "p (w i) -> p w i", i=4)
                                    og = l1[:, lof + lo // 2:
                                            lof + hi // 2].rearrange(
                                        "p (w i) -> p w i", i=4)
                                    nc.vector.tensor_tensor(
                                        out=og, in0=cg[:, :, 0:4], in1=dh,
                                        op=mybir.AluOpType.min,
                                    )
                        if "folds" in ABLATE:
                            nc.vector.memset(
                                wt[:, fb * FB * NWIN * WSTOP:
                                   fb * FB * NWIN * WSTOP + 16], 1.0)
                        elif WSTOP == 2:
                            c1 = l1[:].rearrange("p (w i) -> p w i", i=4)
                            o2 = wt[:, fb * FB * NWIN * WSTOP:
                                    (fb + 1) * FB * NWIN * WSTOP].rearrange(
                                "p (w i) -> p w i", i=WSTOP)
                            eng = nc.gpsimd if FOLD42_POOL else nc.vector
                            eng.tensor_tensor(
                                out=o2, in0=c1[:, :, 0:2], in1=c1[:, :, 2:4],
                                op=mybir.AluOpType.min,
                            )
                    o0 = cu * CHUNK_U * NWIN * WSTOP
                    nc.sync.dma_start(
                        out=winm_out[:, o0:o0 + CHUNK_U * NWIN * WSTOP],
                        in_=wt[:],
                    )
    nc.compile()
    return nc


# ---------------------------------------------------------------------------
# host: candidate construction
# ---------------------------------------------------------------------------
def _kd_order(pts, leaf):
    out = []

    def rec(ids):
        if len(ids) <= leaf:
            out.append(ids)
            return
        P = pts[ids]
        ax = int(np.argmax(P.max(0) - P.min(0)))
        half = len(ids) // 2
        part = np.argpartition(P[:, ax], half)
        rec(ids[part[:half]])
        rec(ids[part[half:]])

    rec(np.arange(len(pts)))
    return np.concatenate(out)


def _build_side(Q, T, qn2, tn2):
    """Candidates for one (batch, direction).
    Returns qperm [N], cand_idx [NQB, C] int64 (-1 pad), fallback list."""
    nq, nt = len(Q), len(T)
    qperm = _kd_order(Q, QBLK)
    tperm = _kd_order(T, CELL)
    Qs = Q[qperm]
    Ts = T[tperm]
    ntc = nt // CELL
    Tc = Ts.reshape(ntc, CELL, 3)
    blo, bhi = Tc.min(1), Tc.max(1)
    cc = Tc.mean(1)
    ub2 = np.full(nq, np.inf, np.float32)
    for s in range(0, nq, 2048):
        q = Qs[s:s + 2048]
        dcc = ((q[:, None] - cc[None]) ** 2).sum(-1)
        ci = np.argpartition(dcc, UB_CELLS - 1, axis=1)[:, :UB_CELLS]
        for kk in range(UB_CELLS):
            pts = Tc[ci[:, kk]]
            ub2[s:s + 2048] = np.minimum(
                ub2[s:s + 2048], ((q[:, None] - pts) ** 2).sum(-1).min(1))
    cand_idx = np.full((NQB, C), -1, np.int64)
    fallback = []
    tcells = tperm.reshape(ntc, CELL)
    for qb in range(NQB):
        q = Qs[qb * QBLK:(qb + 1) * QBLK]
        d = (np.maximum(blo[None] - q[:, None], 0)
             + np.maximum(q[:, None] - bhi[None], 0))
        lb2 = (d ** 2).sum(-1)
        m = (lb2 <= ub2[qb * QBLK:(qb + 1) * QBLK][:, None]).any(0)
        cells = np.nonzero(m)[0]
        if len(cells) > NCELLS:
            fallback.append(qb)
            cells = cells[:NCELLS]
        gi = tcells[cells].reshape(-1)
        cand_idx[qb, :len(gi)] = gi
    return qperm, cand_idx, fallback


def _core_assign(cid):
    return cid // 4, (cid // 2) % 2, cid % 2   # batch, direction, half


def _make_in_maps(pp, rp, pn2, rm2):
    """Candidate construction + packed augmented operands for all cores.
    Returns in_maps plus the per-(b,dir) metadata the host post needs."""
    sides = {}
    for b in range(B):
        for dir_ in range(2):
            Q, T = (rp[b], pp[b]) if dir_ == 0 else (pp[b], rp[b])
            qn2, tn2 = (rm2[b], pn2[b]) if dir_ == 0 else (pn2[b], rm2[b])
            qperm, cand_idx, fb = _build_side(Q, T, qn2, tn2)
            # encode stationary for all 64 blocks at once
            stat = _encode_stationary(Q[qperm], qn2[qperm])   # [18, N]
            # moving: gather candidates
            civ = np.where(cand_idx >= 0, cand_idx, 0)
            pts = T[civ.reshape(-1)]                          # [NQB*C, 3]
            n2 = tn2[civ.reshape(-1)]
            pad = (cand_idx < 0).reshape(-1)
            mov = _encode_moving(pts, n2, pad)                # [18, NQB*C]
            sides[(b, dir_)] = dict(qperm=qperm, cand_idx=cand_idx,
                                    fallback=fb, stat=stat, mov=mov)
    in_maps = []
    for cid in range(NCORES):
        b, dir_, half = _core_assign(cid)
        s = sides[(b, dir_)]
        q0 = half * UNITS
        in_maps.append({
            "pm": np.ascontiguousarray(
                s["mov"][:, q0 * C:(q0 + UNITS) * C]),
            "rs": np.ascontiguousarray(
                s["stat"][:, q0 * QBLK:(q0 + UNITS) * QBLK]),
        })
    return in_maps, sides


# ---------------------------------------------------------------------------
# PJRT runner (identical machinery to the validated baseline)
# ---------------------------------------------------------------------------
def _get_runner(loop_iters=None):
    key = ("runner", loop_iters)
    if key in _cache:
        return _cache[key]
    import concourse.mybir as mybir
    from concourse import bass2jax
    import jax
    from jax.sharding import Mesh, PartitionSpec
    from jax.experimental.shard_map import shard_map

    nc = _build_program(loop_iters)
    bass2jax.install_neuronx_cc_hook()

    partition_name = (nc.partition_id_tensor.name
                      if nc.partition_id_tensor else None)
    in_names, out_names, out_avals = [], [], []
    for alloc in nc.m.functions[0].allocations:
        if not isinstance(alloc, mybir.MemoryLocationSet):
            continue
        name = alloc.memorylocations[0].name
        if alloc.kind == "ExternalInput":
            if name != partition_name:
                in_names.append(name)
        elif alloc.kind == "ExternalOutput":
            out_names.append(name)
            out_avals.append(jax.core.ShapedArray(
                tuple(alloc.tensor_shape), mybir.dt.np(alloc.dtype)))
    n_params = len(in_names)
    all_names = in_names + out_names
    if partition_name is not None:
        all_names = all_names + [partition_name]

    def _body(*args):
        operands = list(args)
        if partition_name is not None:
            operands.append(bass2jax.partition_id_tensor())
        outs = bass2jax._bass_exec_p.bind(
            *operands,
            out_avals=tuple(out_avals),
            in_names=tuple(all_names),
            out_names=tuple(out_names),
            lowering_input_output_aliases=(),
            sim_require_finite=True,
            sim_require_nnan=True,
            nc=nc,
        )
        return tuple(outs)

    devices = jax.devices()[:NCORES]
    mesh = Mesh(np.asarray(devices), ("core",))
    n_outs = len(out_names)
    sharded = jax.jit(
        shard_map(_body, mesh=mesh,
                  in_specs=(PartitionSpec("core"),) * (n_params + n_outs),
                  out_specs=(PartitionSpec("core"),) * n_outs,
                  check_rep=False),
        keep_unused=True,
    )
    zero_outs = [np.zeros((NCORES * a.shape[0], *a.shape[1:]), a.dtype)
                 for a in out_avals]
    runner = {"fn": sharded, "in_names": in_names, "out_names": out_names,
              "out_avals": out_avals, "zero_outs": zero_outs}
    _cache[key] = runner
    return runner


class _Res:
    def __init__(self, results):
        self.results = results
        self.exec_time_ns = None
        self.instructions_and_trace = None


def _run_device(in_maps, trace=False):
    import jax
    r = _get_runner()
    concat_in = [np.concatenate([m[name] for m in in_maps], axis=0)
                 for name in r["in_names"]]
    out_arrs = r["fn"](*concat_in, *r["zero_outs"])
    jax.block_until_ready(out_arrs)
    results = [
        {name: np.asarray(out_arrs[i]).reshape(
            NCORES, *r["out_avals"][i].shape)[c]
         for i, name in enumerate(r["out_names"])}
        for c in range(NCORES)
    ]
    return _Res(results)


def _time_variant(in_maps, loop_iters, n):
    import time
    import jax
    r = _get_runner(loop_iters)
    concat_in = [np.concatenate([m[name] for m in in_maps], axis=0)
                 for name in r["in_names"]]
    dev_in = [jax.device_put(x) for x in concat_in]
    dev_zero = [jax.device_put(z) for z in r["zero_outs"]]
    jax.block_until_ready(dev_in + dev_zero)
    jax.block_until_ready(r["fn"](*dev_in, *dev_zero))  # warmup
    times = []
    for _ in range(n):
        t0 = time.perf_counter()
        jax.block_until_ready(r["fn"](*dev_in, *dev_zero))
        times.append(time.perf_counter() - t0)
    return times


def _time_runs(in_maps, n=12, iters=4096):
    """Interleave the 1-iter and (1+iters)-iter variants so machine slow
    states hit both endpoints alike; min over endpoints cancels dispatch."""
    t1, tk = [], []
    for i in range(n):
        t1.extend(_time_variant(in_maps, 1, 1))
        tk.extend(_time_variant(in_maps, 1 + iters, 1))
    per_iter = (min(tk) - min(t1)) / iters
    return per_iter, t1, tk


# ---------------------------------------------------------------------------
# host post: top-2 window recheck -> exact argmin / min
# ---------------------------------------------------------------------------
def _host_post(Q, T, qn2, tn2, qperm, cand_idx, winm, fallback):
    """winm [NQB, QBLK, NWIN, WSTOP] fp16 -> exact best d2 + argmin."""
    nq = len(Q)
    best = np.empty(nq, np.float32)
    barg = np.empty(nq, np.int64)
    ar = np.arange(WIN)
    mloc = np.arange(QBLK)
    wm_all = winm.min(axis=3).astype(np.float32)      # [NQB, QBLK, NWIN]
    top2 = np.argpartition(wm_all, 1, axis=2)[:, :, :2]   # [NQB, QBLK, 2]
    pos = (top2[..., None] * WIN + ar[None, None, None, :]
           ).reshape(NQB, QBLK, 2 * WIN)
    for qb in range(NQB):
        qi = qperm[qb * QBLK:(qb + 1) * QBLK]
        ci = cand_idx[qb][pos[qb]]                    # [QBLK, 16]
        valid = ci >= 0
        civ = np.where(valid, ci, 0)
        qpts = Q[qi]
        tp = T[civ]
        d2w = (tn2[civ] + qn2[qi][:, None]
               - 2.0 * np.einsum('qwd,qd->qw', tp, qpts, dtype=np.float32))
        d2w = np.maximum(d2w, 0.0)
        d2w = np.where(valid, d2w, np.inf)
        j = np.argmin(d2w, axis=1)
        best[qi] = d2w[mloc, j]
        barg[qi] = civ[mloc, j]
    for qb in fallback:
        qi = qperm[qb * QBLK:(qb + 1) * QBLK]
        d2 = (qn2[qi][:, None] + tn2[None, :] - 2.0 * Q[qi] @ T.T)
        d2 = np.maximum(d2, 0.0)
        barg[qi] = np.argmin(d2, axis=1)
        best[qi] = d2[mloc[:len(qi)], barg[qi]]
    return best, barg


def kernel(**inputs):
    return _kernel_impl(inputs, trace=False)[0]


def _kernel_impl(inputs, trace=False):
    pp = np.asarray(inputs["predicted_points"], np.float32)
    ps_ = np.asarray(inputs["predicted_sdfs"], np.float32)
    pc = np.asarray(inputs["predicted_colors"], np.float32)
    rp = np.asarray(inputs["ref_points"], np.float32)
    rs_ = np.asarray(inputs["ref_sdfs"], np.float32)
    rc = np.asarray(inputs["ref_colors"], np.float32)

    pn2 = (pp * pp).sum(-1)
    rm2 = (rp * rp).sum(-1)

    in_maps, sides = _make_in_maps(pp, rp, pn2, rm2)
    res = _run_device(in_maps, trace=trace)
    outs = res.results

    # stitch device winm back into [NQB, QBLK, NWIN, WSTOP] per (b, dir)
    winms = {}
    for cid in range(NCORES):
        b, dir_, half = _core_assign(cid)
        w = np.asarray(outs[cid]["winm"]).reshape(
            128, UNITS, NWIN, WSTOP).transpose(1, 0, 2, 3)
        winms.setdefault((b, dir_), np.empty(
            (NQB, QBLK, NWIN, WSTOP), np.float16))[
            half * UNITS:(half + 1) * UNITS] = w

    colmin = np.empty((B, M), np.float32)
    closest = np.empty((B, M), np.int64)
    rowmin = np.empty((B, N), np.float32)
    for b in range(B):
        for dir_ in range(2):
            Q, T = (rp[b], pp[b]) if dir_ == 0 else (pp[b], rp[b])
            qn2, tn2 = (rm2[b], pn2[b]) if dir_ == 0 else (pn2[b], rm2[b])
            s = sides[(b, dir_)]
            best, barg = _host_post(Q, T, qn2, tn2, s["qperm"],
                                    s["cand_idx"], winms[(b, dir_)],
                                    s["fallback"])
            if dir_ == 0:
                colmin[b] = best
                closest[b] = barg
            else:
                rowmin[b] = best

    cham_xy = rowmin.mean(axis=1)
    cham_yx = colmin.mean(axis=1)
    chamfer = np.float32((cham_xy + cham_yx).mean())

    bi = np.arange(B)[:, None]
    g_sdfs = rs_[bi, closest, :]
    sdf_l1 = np.float32(np.abs(g_sdfs - ps_).mean())
    g_cols = rc[bi, closest, :]
    color_l1 = np.float32(np.abs(g_cols - pc).mean())

    out = np.stack([sdf_l1, color_l1, chamfer]).astype(np.float32)
    return out, res
